# revision 20
# baseline (speedup 1.0000x reference)
"""Trainium2 Bass kernel for nn_ContextPredictionModel (dense_cnn).

Contract: kernel(**inputs) takes FULL unsharded inputs (numpy), returns the
FULL [120, 256, 1024] f32 output. Internally shards batch B=256 across 8
NeuronCores (data parallel) and syncs BatchNorm statistics with AllReduce.

Math notes (vs reference):
  - conv biases of layers 0 and 1 are channel-constant shifts of the next
    BatchNorm's input, so they cancel exactly in BN -> dropped.
  - layer-2 conv bias + the 1/9 avg-pool factor are folded on the host into
    the prediction-head weights/biases:
        pred = W @ (pool_sum/9 + b2) = (W/9) @ pool_sum + (W @ b2 + lb)
  - the 1x1 conv of layer 2 commutes with the avg-pool (the conv is a
    per-pixel linear map), so the kernel pools the BN+ReLU output over the
    3x3 window FIRST and then runs the conv with free dim bl per patch,
    batching patches chunk-wise into wide matmuls.
  - patch 12 (grid center) feeds no prediction head, and BN statistics are
    strictly per-patch, so its whole pipeline is dropped (24 live patches).
  - prediction heads run "transposed": the head weight tile is the
    stationary operand, pooled contexts are the moving operand, producing
    [out_c, patch*b] tiles; the final per-head transpose happens on host.
  - layer-0 BN statistics depend only on the input x, so the affine
    coefficients a0/d0 are precomputed on the host (input preprocessing);
    layer-1/2 BN stats are computed on device from conv outputs (bn_stats)
    and merged across cores with AllReduces.  Cross-core NEFF launch is
    staggered by tens of us, so each layer's stats sync is split into SIX
    chunks (patch subsets, column-structured so all access patterns stay
    regular) that are triggered as early as possible and consumed as late
    as possible; patches are processed in head-need order (d0's patches
    first, then d3's, d2's, d1's) so every AllReduce hides behind compute.
"""

import os
import numpy as np
import ml_dtypes

import concourse.bass as bass
import concourse.mybir as mybir
import concourse.tile as tile
from concourse import bacc
from concourse import bass_utils

# ---------------- problem constants (hardcoded; self-contained) -------------
B_FULL = 256
C_FULL = 1024
HW = 7
NL = 3
NPATCH = 25
PPAD = 28   # patch-dim padding so strided (g,5)-views stay in bounds
KPIX = 9    # 3x3
NCORES = 8
EPS = 1e-5
NHEADS = 12

# matmul/storage dtype: "bf16" | "f32r" | "f32" (env override for experiments)
DTYPE = os.environ.get("CPM_DTYPE", "bf16")
GROUP = 2  # patches per conv group
TRACE = False  # set True from test harness to capture NTFF profile
LAST_RESULT = None  # BassKernelResults of last kernel() call

_AF = mybir.ActivationFunctionType
_ALU = mybir.AluOpType

# Stats-sync / processing chunks: (patch_offset, grid_rows g, grid_cols f)
# covering ids {off + 5*gi + fi}.  Order = processing order:
#   chunks 0-2 = d0's patches {0..9}, chunk 3 = d3's extras, chunk 4 = d2's
#   extras, chunk 5 = d1's extras.  Patch 12 is dead (no head reads it).
CHUNKS = [
    (0, 2, 2),    # {0,1,5,6}   \
    (2, 2, 2),    # {2,3,7,8}    } d0's patches, split for earlier syncs
    (4, 2, 1),    # {4,9}       /
    (13, 3, 2),   # {13,14,18,19,23,24}  (d3 extras)
    (10, 3, 2),   # {10,11,15,16,20,21}  (d2 extras)
    (17, 2, 1),   # {17,22}  (d1 extras)
]
CHUNK_IDS = [[off + 5 * gi + fi for gi in range(g) for fi in range(f)]
             for (off, g, f) in CHUNKS]
PROC_ORDER = [i for ids in CHUNK_IDS for i in ids]
CHUNK_POS0 = [0]
for ids in CHUNK_IDS:
    CHUNK_POS0.append(CHUNK_POS0[-1] + len(ids))
NLIVE = CHUNK_POS0[-1]  # 24
# head directions in ctx-chunk readiness order: d0, d3, d2, d1
DIR_HEADS = [(0, 1, 2), (9, 10, 11), (6, 7, 8), (3, 4, 5)]


def _pred_index_map():
    """m[h, i] = row in the final [120, B, C] output for the i-th
    (ascending-p) patch of head h (h = d*3 + s)."""
    m = np.zeros((NHEADS, 10), dtype=np.int64)
    cnt = [0] * NHEADS
    j = 0
    for y1 in range(5):
        for x1 in range(5):
            conds = []
            if y1 + 2 in (2, 3):
                conds.append(0)
            if y1 in (3, 4):
                conds.append(1)
            if x1 + 2 in (2, 3):
                conds.append(2)
            if x1 in (3, 4):
                conds.append(3)
            for d in conds:
                for s in range(3):
                    h = d * 3 + s
                    m[h, cnt[h]] = j
                    cnt[h] += 1
                    j += 1
    assert j == 120 and all(c == 10 for c in cnt)
    return m


def _dt_pair(dt_str):
    if dt_str == "bf16":
        return mybir.dt.bfloat16, ml_dtypes.bfloat16
    if dt_str == "f32r":
        return mybir.dt.float32r, np.float32
    if dt_str == "f32":
        return mybir.dt.float32, np.float32
    raise ValueError(dt_str)


def build_nc(ncores=NCORES, bl=B_FULL // NCORES, c=C_FULL, dt_str=DTYPE,
             group=GROUP):
    """Build + compile the per-core Bass program (SPMD, same on all cores)."""
    D, _ = _dt_pair(dt_str)
    f32 = mybir.dt.float32
    nct = c // 128             # channel tiles
    nact = bl * KPIX           # conv matmul free dim per patch
    nhalf = nact // 2          # bn_stats even/odd half count
    ntot = ncores * bl * KPIX  # global BN count per (patch, channel)
    nrows = 10 * bl            # head free dim (10 patches x bl)
    # conv patch groups (weight reuse across patches within a group)
    pgroups = [PROC_ORDER[i:i + group] for i in range(0, NLIVE, group)]
    thresholds = {CHUNK_POS0[k + 1]: k for k in range(len(CHUNKS))}

    nc = bacc.Bacc("TRN2", target_bir_lowering=False, debug=False,
                   num_devices=ncores)

    # ---------------- I/O ----------------
    x_in = nc.dram_tensor("x_t", [c, bl, HW * HW], D, kind="ExternalInput")
    cw_in = nc.dram_tensor("cw_t", [NL, c, c], D, kind="ExternalInput")
    lw_in = nc.dram_tensor("lw_t", [NHEADS, c, c], D, kind="ExternalInput")
    gam_in = nc.dram_tensor("gam_t", [NL, c], f32, kind="ExternalInput")
    bet_in = nc.dram_tensor("bet_t", [NL, c], f32, kind="ExternalInput")
    a0_in = nc.dram_tensor("a0_t", [c, NPATCH], f32, kind="ExternalInput")
    d0_in = nc.dram_tensor("d0_t", [c, NPATCH], f32, kind="ExternalInput")
    # transposed head outputs: [head, out_c tile, out_c in tile, patch*b]
    preds_out = nc.dram_tensor("preds_t", [NHEADS, nct, 128, nrows], f32,
                               kind="ExternalOutput")

    # internal DRAM: streamed activations + collective bounce buffers
    h_dram = [nc.dram_tensor(f"h{l}", [NPATCH, nct, 128, nact], D)
              for l in range(2)]
    cc_bufs = {}
    for l in (1, 2):
        for ci, ids in enumerate(CHUNK_IDS):
            n = len(ids)
            cc_bufs[(l, ci, "in")] = nc.dram_tensor(
                f"cc_in{l}_{ci}", [128, nct * n * 2], f32)
            cc_bufs[(l, ci, "out")] = nc.dram_tensor(
                f"cc_out{l}_{ci}", [128, nct * n * 2], f32,
                addr_space="Shared")

    patches = [(y, x) for y in range(5) for x in range(5)]

    with tile.TileContext(nc) as tc:
        import contextlib
        with contextlib.ExitStack() as ctx:
            const = ctx.enter_context(tc.tile_pool(name="const", bufs=1))
            statsp = ctx.enter_context(tc.tile_pool(name="stats", bufs=1))
            coefp = ctx.enter_context(tc.tile_pool(name="coef", bufs=8))
            psp = ctx.enter_context(
                tc.tile_pool(name="ps", bufs=8, space="PSUM"))

            # ---------------- constants ----------------
            # conv weights for layers 0/1; layer-2's live in the pool slot
            # freed by x after layer 0 (cw2p below)
            cw_sb = const.tile([128, 2, nct, c], D)
            gam_sb = const.tile([128, NL, nct], f32)
            bet_sb = const.tile([128, NL, nct], f32)
            eps_sb = const.tile([128, 1], f32)
            nc.vector.memset(eps_sb[:], EPS)
            # pooled contexts [in_c, patch(id), b]; layer-2 rhs by position
            ctx_sb = const.tile([128, nct, PPAD, bl], D)
            rhs2_sb = const.tile([128, nct, NLIVE * bl], D)

            # per-layer BN affine coefs; layer 0 comes from the host
            coef_a = [coefp.tile([128, nct, PPAD], f32, tag="cf",
                                 name=f"coefa{i}") for i in range(3)]
            coef_d = [coefp.tile([128, nct, PPAD], f32, tag="cf",
                                 name=f"coefd{i}") for i in range(3)]
            # raw bn_stats 6-tuples for layers 1,2 (filled by conv epilogues)
            bnst = {l: statsp.tile([128, nct, PPAD, 6], f32, tag=f"st{l}",
                                   name=f"bnst{l}") for l in (1, 2)}

            def cview(ap_5g, g, f):
                """[..., 5g] patch-window view -> [..., g, f]."""
                return ap_5g.rearrange("p c (g r) -> p c g r", g=g)[
                    :, :, :, 0:f]

            st_tiles = {}

            def emit_stats_push(l, ci):
                """Convert chunk's bn_stats -> (sum,sumsq) and AllReduce.
                Emitted as early as the chunk's conv outputs allow."""
                off, g, f = CHUNKS[ci]
                n = g * f
                sl = slice(off, off + 5 * g)
                me = cview(bnst[l][:, :, sl, 1], g, f)
                M2e = cview(bnst[l][:, :, sl, 2], g, f)
                mo = cview(bnst[l][:, :, sl, 4], g, f)
                M2o = cview(bnst[l][:, :, sl, 5], g, f)
                st = statsp.tile([128, nct, n, 2], f32, tag=f"ss{ci}",
                                 name=f"ss{l}_{ci}")
                st_tiles[(l, ci)] = st
                stv = st[:].rearrange("p c (g f) s -> p c g f s", g=g)
                t1 = coefp.tile([128, nct, g, f], f32, tag="cvt",
                                name=f"cvt1_{l}{ci}")
                t2 = coefp.tile([128, nct, g, f], f32, tag="cvt",
                                name=f"cvt2_{l}{ci}")
                t3 = coefp.tile([128, nct, g, f], f32, tag="cvt",
                                name=f"cvt3_{l}{ci}")
                # sum = nhalf * (me + mo)
                nc.vector.tensor_tensor(t1[:], me, mo, _ALU.add)
                nc.vector.tensor_scalar_mul(stv[:, :, :, :, 0], t1[:],
                                            float(nhalf))
                # sumsq = (M2e + M2o) + nhalf * (me^2 + mo^2)
                nc.vector.tensor_tensor(t2[:], me, me, _ALU.mult)
                nc.vector.tensor_tensor(t3[:], mo, mo, _ALU.mult)
                nc.vector.tensor_tensor(t2[:], t2[:], t3[:], _ALU.add)
                nc.vector.tensor_tensor(t3[:], M2e, M2o, _ALU.add)
                nc.vector.scalar_tensor_tensor(
                    out=stv[:, :, :, :, 1], in0=t2[:], scalar=float(nhalf),
                    in1=t3[:], op0=_ALU.mult, op1=_ALU.add)
                # AllReduce
                flat = st[:].rearrange("p a b c -> p (a b c)")
                nc.gpsimd.dma_start(out=cc_bufs[(l, ci, "in")][:], in_=flat)
                nc.gpsimd.collective_compute(
                    "AllReduce", _ALU.add,
                    replica_groups=[list(range(ncores))],
                    ins=[cc_bufs[(l, ci, "in")][:].opt()],
                    outs=[cc_bufs[(l, ci, "out")][:].opt()])
                nc.gpsimd.dma_start(out=flat,
                                    in_=cc_bufs[(l, ci, "out")][:])

            def emit_stats_finish(l, ci):
                """Sqrt + BN affine coef math for a synced chunk.  Emitted
                LATE (just before the coefs' consumers) so the mesh wait
                doesn't clog the scalar/vector queues mid-layer."""
                off, g, f = CHUNKS[ci]
                sl = slice(off, off + 5 * g)
                st = st_tiles[(l, ci)]
                stv = st[:].rearrange("p c (g f) s -> p c g f s", g=g)
                m_t = coefp.tile([128, nct, g, f], f32, tag="cvt",
                                 name=f"m_{l}{ci}")
                v_t = coefp.tile([128, nct, g, f], f32, tag="cvt",
                                 name=f"v_{l}{ci}")
                a_t = cview(coef_a[l][:, :, sl], g, f)
                d_t = cview(coef_d[l][:, :, sl], g, f)
                gbc = gam_sb[:, l, :, None, None].to_broadcast(
                    (128, nct, g, f))
                bbc = bet_sb[:, l, :, None, None].to_broadcast(
                    (128, nct, g, f))
                inv_n = 1.0 / float(ntot)
                nc.vector.tensor_scalar_mul(m_t[:], stv[:, :, :, :, 0],
                                            inv_n)
                nc.vector.tensor_tensor(v_t[:], m_t[:], m_t[:], _ALU.mult)
                nc.vector.scalar_tensor_tensor(
                    out=v_t[:], in0=stv[:, :, :, :, 1], scalar=inv_n,
                    in1=v_t[:], op0=_ALU.mult, op1=_ALU.subtract)
                nc.scalar.activation(out=v_t[:], in_=v_t[:], func=_AF.Sqrt,
                                     bias=eps_sb[:], scale=1.0)
                nc.vector.reciprocal(out=v_t[:], in_=v_t[:])
                # a = gamma * rstd ; d = beta - mean * a
                nc.vector.tensor_tensor(a_t, v_t[:], gbc, _ALU.mult)
                nc.vector.tensor_tensor(d_t, m_t[:], a_t, _ALU.mult)
                nc.vector.tensor_tensor(d_t, bbc, d_t, _ALU.subtract)

            # conv + head pools.  l2ap/lwp are opened in the OUTER scope so
            # their tiles carry no write-after-read dependency on the conv
            # pools (which would stall layer-2 work until layer 1 finished).
            # xp is innermost: its slot is reused for layer-2 conv weights.
            with tc.tile_pool(name="raw", bufs=3) as rawp, \
                 tc.tile_pool(name="hs", bufs=2) as hsp, \
                 tc.tile_pool(name="l2a", bufs=2) as l2ap, \
                 tc.tile_pool(name="lwp", bufs=3) as lwp:

                def emit_layer(l, x_sb, deferred=None):
                    a_t, d_t = coef_a[l], coef_d[l]
                    done = 0
                    for pg in pgroups:
                        # layer>=1: BN coefs come from a synced chunk; emit
                        # the (mesh-waiting) finish just before first use
                        if l > 0 and done in dict(
                                (CHUNK_POS0[k], k) for k in
                                range(len(CHUNKS))):
                            emit_stats_finish(
                                l, [k for k in range(len(CHUNKS))
                                    if CHUNK_POS0[k] == done][0])
                        rhs_t, stage_t = {}, {}
                        for p in pg:
                            y, x0 = patches[p]
                            if l > 0:
                                raw = rawp.tile([128, nct, nact], D,
                                                tag="raw", name=f"raw{p}")
                                nc.sync.dma_start(
                                    out=raw[:],
                                    in_=h_dram[l - 1][p].rearrange(
                                        "c q n -> q c n"))
                            rhs = rhsp.tile([128, nct, nact], D, tag="rhs",
                                            name=f"rhs{p}")
                            rhs_t[p] = rhs
                            for ct in range(nct):
                                if l == 0:
                                    xin = x_sb[:, ct].rearrange(
                                        "p b (h w) -> p b h w", w=HW)[
                                        :, :, y:y + 3, x0:x0 + 3]
                                    rout = rhs[:, ct].rearrange(
                                        "p (b h w) -> p b h w", b=bl, h=3)
                                else:
                                    xin = raw[:, ct]
                                    rout = rhs[:, ct]
                                nc.scalar.activation(
                                    out=rout, in_=xin, func=_AF.Relu,
                                    scale=a_t[:, ct, p:p + 1],
                                    bias=d_t[:, ct, p:p + 1])
                            stage_t[p] = stgp.tile(
                                [128, nct, nact], D, tag="stg",
                                name=f"stg{p}")
                        if deferred is not None:
                            deferred()
                            deferred = None
                        pouts = {}
                        for ot in range(nct):
                            for p in pg:
                                pouts[p] = psp.tile([128, 512], f32,
                                                    tag="ps",
                                                    name=f"ps{p}_{ot}")
                            for ct in range(nct):
                                for p in pg:
                                    nc.tensor.matmul(
                                        pouts[p][:, :nact],
                                        cw_sb[:, l, ct,
                                              ot * 128:(ot + 1) * 128],
                                        rhs_t[p][:, ct],
                                        start=(ct == 0),
                                        stop=(ct == nct - 1))
                            for p in pg:
                                pout = pouts[p][:, :nact]
                                nc.vector.bn_stats(
                                    out=bnst[l + 1][:, ot, p, :],
                                    in_=pout)
                                nc.vector.tensor_copy(
                                    out=stage_t[p][:, ot], in_=pout)
                        for p in pg:
                            nc.scalar.dma_start(
                                out=h_dram[l][p].rearrange("c q n -> q c n"),
                                in_=stage_t[p][:])
                        done += len(pg)
                        if done in thresholds:
                            emit_stats_push(l + 1, thresholds[done])

                with tc.tile_pool(name="rhs", bufs=3) as rhsp, \
                     tc.tile_pool(name="stg", bufs=2) as stgp:
                    with tc.tile_pool(name="xp", bufs=1) as xp:
                        # small coef/bn tensors first (first acts need them)
                        nc.gpsimd.dma_start(
                            out=gam_sb[:],
                            in_=gam_in[:].rearrange("l (ct p) -> p l ct",
                                                    p=128))
                        nc.gpsimd.dma_start(
                            out=bet_sb[:],
                            in_=bet_in[:].rearrange("l (ct p) -> p l ct",
                                                    p=128))
                        nc.gpsimd.dma_start(
                            out=coef_a[0][:, :, 0:NPATCH],
                            in_=a0_in[:].rearrange("(ct p) q -> p ct q",
                                                   p=128))
                        nc.gpsimd.dma_start(
                            out=coef_d[0][:, :, 0:NPATCH],
                            in_=d0_in[:].rearrange("(ct p) q -> p ct q",
                                                   p=128))
                        # x next (gates the first matmuls), layer-0 weights
                        # in parallel on the scalar queue; layer-1/2 weights
                        # deferred until layer 0 is underway
                        x_sb = xp.tile([128, nct, bl, HW * HW], D)
                        xr = x_in[:].rearrange("(ct p) b x -> p ct b x",
                                               p=128)
                        for ct in range(nct):
                            nc.sync.dma_start(out=x_sb[:, ct], in_=xr[:, ct])
                        cwr = cw_in[:].rearrange("l (ct p) o -> p l ct o",
                                                 p=128)
                        for ct in range(nct):
                            nc.scalar.dma_start(out=cw_sb[:, 0, ct],
                                                in_=cwr[:, 0, ct])

                        def load_l1_cw():
                            for ct in range(nct):
                                nc.scalar.dma_start(out=cw_sb[:, 1, ct],
                                                    in_=cwr[:, 1, ct])

                        emit_layer(0, x_sb, deferred=load_l1_cw)

                    # layer-2 conv weights into the slot x just freed
                    with tc.tile_pool(name="cw2p", bufs=1) as cw2p:
                        cw2_sb = cw2p.tile([128, nct, c], D)
                        for ct in range(nct):
                            nc.scalar.dma_start(out=cw2_sb[:, ct],
                                                in_=cwr[:, 2, ct])
                        emit_layer(1, None)

                        # ---- layer 2: BN+ReLU, pool 3x3, 1x1 conv, heads --
                        lw_tiles = {}

                        def emit_l2_pool(ci):
                            emit_stats_finish(2, ci)
                            a_t, d_t = coef_a[2], coef_d[2]
                            for pos_off, p in enumerate(CHUNK_IDS[ci]):
                                pos = CHUNK_POS0[ci] + pos_off
                                raw = rawp.tile([128, nct, nact], D,
                                                tag="raw", name=f"raw2_{p}")
                                nc.sync.dma_start(
                                    out=raw[:],
                                    in_=h_dram[1][p].rearrange(
                                        "c q n -> q c n"))
                                act = l2ap.tile([128, nct, nact], D,
                                                tag="act", name=f"act{p}")
                                for ct in range(nct):
                                    nc.scalar.activation(
                                        out=act[:, ct], in_=raw[:, ct],
                                        func=_AF.Relu,
                                        scale=a_t[:, ct, p:p + 1],
                                        bias=d_t[:, ct, p:p + 1])
                                with nc.allow_low_precision(
                                        reason="pool-sum to mm dtype"):
                                    nc.vector.tensor_reduce(
                                        out=rhs2_sb[:, :,
                                                    pos * bl:(pos + 1) * bl],
                                        in_=act[:].rearrange(
                                            "p c (b x) -> p c b x", x=KPIX),
                                        axis=mybir.AxisListType.X,
                                        op=_ALU.add)

                        def emit_lw_loads(heads):
                            # split each load across the sync and scalar
                            # queues so head weights stream 2x as fast
                            for h in heads:
                                lw_sb = lwp.tile([128, nct, c], D, tag="lw",
                                                 name=f"lw{h}")
                                lw_tiles[h] = lw_sb
                                lwr = lw_in[h].rearrange(
                                    "(ct p) o -> p ct o", p=128)
                                half = nct // 2
                                nc.sync.dma_start(
                                    out=lw_sb[:, 0:half], in_=lwr[:, 0:half])
                                nc.scalar.dma_start(
                                    out=lw_sb[:, half:nct],
                                    in_=lwr[:, half:nct])

                        def emit_l2_mms(ci):
                            off, g, f = CHUNKS[ci]
                            n = g * f * bl
                            lo = CHUNK_POS0[ci] * bl
                            for ot in range(nct):
                                ps = psp.tile([128, 512], f32, tag="ps",
                                              name=f"l2ps{ot}_{ci}")
                                for ct in range(nct):
                                    nc.tensor.matmul(
                                        ps[:, :n],
                                        cw2_sb[:, ct,
                                               ot * 128:(ot + 1) * 128],
                                        rhs2_sb[:, ct, lo:lo + n],
                                        start=(ct == 0),
                                        stop=(ct == nct - 1))
                                nc.vector.tensor_copy(
                                    out=ctx_sb[:, ot, off:off + 5 * g, :]
                                    .rearrange("p (g r) b -> p g r b", g=g)[
                                        :, :, 0:f, :],
                                    in_=ps[:, :n].rearrange(
                                        "p (g f b) -> p g f b", g=g, b=bl))

                        def emit_head(h):
                            d = h // 3
                            lw_sb = lw_tiles[h]
                            for ot in range(nct):
                                ps = psp.tile([128, 512], f32, tag="ps",
                                              name=f"hps{h}_{ot}")
                                for ct in range(nct):
                                    if d == 0:
                                        rv = ctx_sb[:, ct, 0:10, :]
                                    elif d == 1:
                                        rv = ctx_sb[:, ct, 15:25, :]
                                    else:
                                        e0 = 0 if d == 2 else 3
                                        rv = ctx_sb[:, ct, 0:25, :].rearrange(
                                            "q (g r) b -> q g r b", g=5)[
                                            :, :, e0:e0 + 2, :]
                                    nc.tensor.matmul(
                                        ps[:, :nrows],
                                        lw_sb[:, ct,
                                              ot * 128:(ot + 1) * 128],
                                        rv, start=(ct == 0),
                                        stop=(ct == nct - 1))
                                hstage = hsp.tile([128, nrows], f32,
                                                  tag="hs",
                                                  name=f"hst{h}_{ot}")
                                nc.scalar.copy(out=hstage[:],
                                               in_=ps[:, :nrows])
                                nc.sync.dma_start(out=preds_out[h, ot],
                                                  in_=hstage[:])

                        # interleave: direction k's heads are emitted after
                        # the NEXT chunk's acts so the (matmul-gated) head
                        # copies never clog the scalar queue ahead of acts;
                        # lw loads go out one chunk before their heads
                        lw_at = {2: DIR_HEADS[0], 3: DIR_HEADS[1],
                                 4: DIR_HEADS[2], 5: DIR_HEADS[3]}
                        heads_at = {3: DIR_HEADS[0], 4: DIR_HEADS[1],
                                    5: DIR_HEADS[2]}
                        for ci in range(len(CHUNKS)):
                            emit_l2_pool(ci)
                            if ci in lw_at:
                                emit_lw_loads(lw_at[ci])
                            if ci in heads_at:
                                for h in heads_at[ci]:
                                    emit_head(h)
                            emit_l2_mms(ci)
                        for h in DIR_HEADS[3]:
                            emit_head(h)

    nc.compile()
    return nc


# ---------------- host side ----------------
_built = {}


def _get_nc(key, **kw):
    if key not in _built:
        _built[key] = build_nc(**kw)
    return _built[key]


def _host_prep(x, bn_gamma, bn_beta, conv_w, conv_b, lin_w, lin_b,
               ncores, dt_str):
    _, np_dt = _dt_pair(dt_str)
    B, C = x.shape[0], x.shape[1]
    bl = B // ncores
    x = np.ascontiguousarray(np.asarray(x, dtype=np.float32))
    bn_gamma = np.asarray(bn_gamma, dtype=np.float32)
    bn_beta = np.asarray(bn_beta, dtype=np.float32)
    conv_w = np.asarray(conv_w, dtype=np.float32)
    conv_b = np.asarray(conv_b, dtype=np.float32)
    lin_w = np.asarray(lin_w, dtype=np.float32)
    lin_b = np.asarray(lin_b, dtype=np.float32)

    cw_t = np.ascontiguousarray(conv_w.transpose(0, 2, 1)).astype(np_dt)
    lw_eff = np.zeros((NHEADS, C, C), dtype=np.float32)
    lb_eff = np.zeros((NHEADS, C), dtype=np.float32)
    for d in range(4):
        for s in range(3):
            h = d * 3 + s
            lw_eff[h] = lin_w[d, s].T / 9.0
            lb_eff[h] = lin_b[d, s] + lin_w[d, s] @ conv_b[2]
    lw_t = lw_eff.astype(np_dt)

    # layer-0 BN affine coefs from global input statistics (host-side
    # input preprocessing; per-pixel sums shared across overlapping patches)
    xr = x.reshape(B, C, HW, HW).astype(np.float64)
    s_pix = xr.sum(axis=0)            # [C, 7, 7]
    q_pix = (xr * xr).sum(axis=0)     # [C, 7, 7]
    ntot = B * KPIX
    a0 = np.zeros((NPATCH, C), dtype=np.float32)
    d0 = np.zeros((NPATCH, C), dtype=np.float32)
    p = 0
    for y in range(5):
        for x0 in range(5):
            s = s_pix[:, y:y + 3, x0:x0 + 3].sum(axis=(1, 2))
            q = q_pix[:, y:y + 3, x0:x0 + 3].sum(axis=(1, 2))
            mean = s / ntot
            var = q / ntot - mean * mean
            a = bn_gamma[0] / np.sqrt(var + EPS)
            a0[p] = a.astype(np.float32)
            d0[p] = (bn_beta[0] - mean * a).astype(np.float32)
            p += 1

    xf = x.reshape(B, C, HW * HW)
    in_maps = []
    for cid in range(ncores):
        x_t = np.ascontiguousarray(
            xf[cid * bl:(cid + 1) * bl].transpose(1, 0, 2)).astype(np_dt)
        in_maps.append(dict(x_t=x_t, cw_t=cw_t, lw_t=lw_t,
                            gam_t=bn_gamma, bet_t=bn_beta,
                            a0_t=np.ascontiguousarray(a0.T),
                            d0_t=np.ascontiguousarray(d0.T)))
    return in_maps, bl, lb_eff


def kernel(x, bn_gamma, bn_beta, conv_w, conv_b, lin_w, lin_b):
    global LAST_RESULT
    B, C = int(x.shape[0]), int(x.shape[1])
    ncores = NCORES
    bl = B // ncores
    nc = _get_nc((ncores, bl, C, DTYPE), ncores=ncores, bl=bl, c=C,
                 dt_str=DTYPE)
    in_maps, bl, lb_eff = _host_prep(x, bn_gamma, bn_beta, conv_w, conv_b,
                                     lin_w, lin_b, ncores, DTYPE)
    res = bass_utils.run_bass_kernel_spmd(
        nc, in_maps, core_ids=list(range(ncores)), trace=TRACE)
    LAST_RESULT = res
    jmap = _pred_index_map()
    nct = C // 128
    out = np.empty((120, B, C), dtype=np.float32)
    for cid in range(ncores):
        ph = res.results[cid]["preds_t"]  # [12, nct, 128, 10*bl]
        for h in range(NHEADS):
            v = ph[h].reshape(nct, 128, 10, bl)
            v = v.transpose(2, 3, 0, 1).reshape(10, bl, C)
            out[jmap[h], cid * bl:(cid + 1) * bl, :] = v + lb_eff[h]
    return out
